# revision 52
# baseline (speedup 1.0000x reference)
"""Trainium2 Bass kernel for nn_DiversityUncertainty (retrieval_knn).

out = lambda * norm01(entropy(pred)) + norm01(min_l2_dist(U_z, L_z))

Sharding: U_z / pred row-sharded across 8 cores (2048 rows each), L_z
replicated.  Per core:

  distance (orientation: n on partitions, m on free dim):
    - fp16 GEMM  psum[n, m] = -2 * (L @ U^T) chunk   (PE, fp32 accum)
    - ScalarE evacuation  ev = relu((C - |l|^2) - psum)  -- the per-
      partition bias folds the |l|^2 term in; relu only clips values
      that are far from the row minimum (C = 256 ~ E[|l|^2])
    - DVE max-accumulate over the 64 n-chunks (max ev  <=>  min dist^2)
    - PE-transpose + free-dim reduce for the partition-axis max
    - d^2 = (C + |u|^2) - max;  sqrt + one Newton step
  entropy: ACT exp + DVE mul/reduce-add
  global min/max: one 4-scalar AllReduce(max) with negation packing,
  then on-device normalize + combine.

Self-contained: all shapes hardcoded; no sibling imports.
"""

import numpy as np

# ---- problem constants (hardcoded per contract) ----
N_U, N_L, NZ, C = 16384, 8192, 256, 1000
CORES = 8
MU = N_U // CORES          # 2048 rows of U / pred per core
P = 128                    # partitions
MT = MU // P               # 16 m-tiles per core
NCH = N_L // P             # 64 n-chunks of 128
MMN = 512                  # moving free dim per matmul (1 psum bank fp32)
EPS = 1e-18
L2C = 256.0                # centering constant for the l2 bias
FINF = 3.0e38

_CACHE = {}


def _build(lam: float, legalize: bool = True):
    import concourse.bass as bass
    import concourse.tile as tile
    from concourse import mybir

    f32 = mybir.dt.float32
    f16 = mybir.dt.float16
    AX = mybir.AxisListType
    OP = mybir.AluOpType
    AF = mybir.ActivationFunctionType

    nc = bass.Bass(num_devices=CORES)

    # fp16 GEMM operands (fp32/fp32r matmuls run with a ~300ns self-load
    # per matmul; fp16 gets separate FWL weight loads).
    ut_h = nc.declare_dram_parameter("ut", [NZ, MU], f16, isOutput=False)    # -2*U.T
    lt_h = nc.declare_dram_parameter("lt", [NZ, N_L], f16, isOutput=False)   # L.T
    l2b_h = nc.declare_dram_parameter("l2b", [P, NCH], f32, isOutput=False)  # C-|l|^2
    u2_h = nc.declare_dram_parameter("u2c", [P, MT], f32, isOutput=False)    # C+|u|^2
    id_h = nc.declare_dram_parameter("idm", [P, P], f16, isOutput=False)     # identity
    id32_h = nc.declare_dram_parameter("idm32", [P, P], f32, isOutput=False)
    pr_h = nc.declare_dram_parameter("pred", [MU, C], f32, isOutput=False)
    out_h = nc.declare_dram_parameter("outv", [P, MT], f32, isOutput=True)

    cc_in = nc.dram_tensor("cc_in", [4], f32)
    cc_out = nc.dram_tensor("cc_out", [4], f32, addr_space="Shared")
    st_dram = nc.dram_tensor("st_dram", [P, 4], f32)

    from contextlib import ExitStack
    with tile.TileContext(nc) as tc, ExitStack() as stk:
        consts = stk.enter_context(tc.tile_pool(name="consts", bufs=1))
        preds = stk.enter_context(tc.tile_pool(name="preds", bufs=3))
        psums = stk.enter_context(tc.tile_pool(name="psums", bufs=2, space="PSUM"))
        evs = stk.enter_context(tc.tile_pool(name="evs", bufs=4))
        small = stk.enter_context(tc.tile_pool(name="small", bufs=1))

        # ---- resident SBUF tensors ----
        ut0c = [consts.tile([P, MMN], f16, tag=f"ut0_{q}", name=f"ut0_{q}") for q in range(4)]
        ut1c = [consts.tile([P, MMN], f16, tag=f"ut1_{q}", name=f"ut1_{q}") for q in range(4)]
        LTW = N_L // 8
        lt0c = [consts.tile([P, LTW], f16, tag=f"lt0_{q}", name=f"lt0_{q}") for q in range(8)]
        lt1c = [consts.tile([P, LTW], f16, tag=f"lt1_{q}", name=f"lt1_{q}") for q in range(8)]
        l2b = consts.tile([P, NCH], f32, tag="l2b")
        u2s = consts.tile([P, MT], f32, tag="u2s")
        idm = consts.tile([P, P], f16, tag="idm")
        idm32 = consts.tile([P, P], f32, tag="idm32")
        acc = consts.tile([P, MU], f16, tag="acc")      # running max over n-chunks

        S = small.tile([P, MT], f32, tag="S")           # sum(exp(x)*x) per row
        maxT = small.tile([P, MT], f32, tag="maxT")

        # first-needed operands first: each sem lane's FIFO head is what
        # the first n-chunk's matmuls actually wait on
        nc.sync.dma_start(out=lt0c[0], in_=lt_h[0:P, 0:LTW])
        for q in range(4):
            nc.sync.dma_start(out=ut0c[q], in_=ut_h[0:P, q * MMN:(q + 1) * MMN])
        nc.sync.dma_start(out=lt1c[0], in_=lt_h[P:NZ, 0:LTW])
        for q in range(4):
            nc.sync.dma_start(out=ut1c[q], in_=ut_h[P:NZ, q * MMN:(q + 1) * MMN])
        nc.sync.dma_start(out=l2b, in_=l2b_h[:])
        nc.sync.dma_start(out=lt0c[1], in_=lt_h[0:P, LTW:2 * LTW])
        nc.sync.dma_start(out=lt1c[1], in_=lt_h[P:NZ, LTW:2 * LTW])
        nc.sync.dma_start(out=u2s, in_=u2_h[:])
        nc.sync.dma_start(out=idm, in_=id_h[:])
        nc.sync.dma_start(out=idm32, in_=id32_h[:])

        # ---- distance: acc[p, m] = max over n-chunks of
        #      (C - l2[n]) - (-2 u.l)  with n = 128*chunk + p ----
        from concourse.tile import add_dep_helper
        chunk_mm = {}
        for nb in range(NCH):
            ps = psums.tile([P, MU], f32, tag="ps")     # 4 banks
            first_mm = None
            for k in range(2):
                ltkc = lt0c if k == 0 else lt1c
                utkc = ut0c if k == 0 else ut1c
                lo = (nb % 8) * P
                w = ltkc[nb // 8][:, lo:lo + P]
                for s in range(MU // MMN):
                    mm = nc.tensor.matmul(
                        ps[:, s * MMN:(s + 1) * MMN], w,
                        utkc[s],
                        start=(k == 0), stop=(k == 1))
                    if first_mm is None:
                        first_mm = mm
                        chunk_mm[nb] = mm
            # stream in the lt chunk needed 16 n-chunks from now, gated on
            # this chunk's first matmul so early DMA bandwidth goes to the
            # operands needed first
            if nb % 8 == 0 and nb // 8 + 2 < 8:
                q = nb // 8 + 2
                qs = slice(q * LTW, (q + 1) * LTW)
                d0 = nc.sync.dma_start(out=lt0c[q], in_=lt_h[0:P, qs])
                d1 = nc.sync.dma_start(out=lt1c[q], in_=lt_h[P:NZ, qs])
                add_dep_helper(d0.ins, first_mm.ins, sync=True,
                               reason="stage lt behind compute")
                add_dep_helper(d1.ins, first_mm.ins, sync=True,
                               reason="stage lt behind compute")
            if nb == 0:
                nc.scalar.activation(acc, ps, AF.Relu,
                                     bias=l2b[:, 0:1], scale=-1.0)
            elif nb % 8 == 7:
                ev = evs.tile([P, MU], f16, tag="ev")
                nc.vector.tensor_scalar(
                    out=ev, in0=ps, scalar1=l2b[:, nb:nb + 1],
                    scalar2=-1.0, op0=OP.subtract, op1=OP.mult)
                nc.vector.tensor_tensor(out=acc, in0=acc, in1=ev, op=OP.max)
            else:
                ev = evs.tile([P, MU], f16, tag="ev")
                nc.scalar.activation(ev, ps, AF.Relu,
                                     bias=l2b[:, nb:nb + 1], scale=-1.0)
                nc.vector.tensor_tensor(out=acc, in0=acc, in1=ev, op=OP.max)

        # ---- entropy: S[p, t] = sum_c exp(x)*x for row 128*t + p ----
        for t in range(MT):
            pt = preds.tile([P, C], f32, tag="pt")
            pd = nc.sync.dma_start(out=pt, in_=pr_h[t * P:(t + 1) * P, :])
            if t >= 1:
                gate = chunk_mm[min(3 * t, 48)]
                add_dep_helper(pd.ins, gate.ins, sync=True,
                               reason="stage pred behind distance")
            et = preds.tile([P, C], f32, tag="et")
            nc.scalar.activation(et, pt, AF.Exp)
            xe = preds.tile([P, C], f32, tag="xe")
            nc.vector.tensor_mul(xe, et, pt)
            nc.vector.tensor_reduce(
                out=S[:, t:t + 1], in_=xe, axis=AX.X, op=OP.add)


        # prefetch the sqrt table set (evicts exp's set, so only after the
        # last exp); hides the ~2.7us ACT_TABLE_LOAD under distance chunks
        sq_warm = small.tile([P, 1], f32, tag="sq_warm")
        nc.scalar.activation(sq_warm, idm32[:, 0:1], AF.Sqrt)

        # entropy stats early (tail off the critical path)
        sneg = small.tile([P, MT], f32, tag="sneg")
        nc.vector.tensor_scalar_mul(sneg, S, -1.0)
        STp = small.tile([P, 4], f32, tag="STp")
        nc.vector.tensor_reduce(out=STp[:, 0:1], in_=S, axis=AX.X, op=OP.max)
        nc.vector.tensor_reduce(out=STp[:, 1:2], in_=sneg, axis=AX.X, op=OP.max)


        # ---- partition-axis max via PE transpose, then d = sqrt(d^2) ----
        for j in range(MT):
            tps = psums.tile([P, P], f16, tag="ps")
            nc.tensor.transpose(tps, acc[:, j * P:(j + 1) * P], idm)
            nc.vector.tensor_reduce(
                out=maxT[:, j:j + 1], in_=tps, axis=AX.X, op=OP.max)

        d2 = small.tile([P, MT], f32, tag="d2")
        nc.vector.tensor_sub(d2, u2s, maxT)             # (C+u2) - max = min d^2
        nc.vector.tensor_scalar_max(d2, d2, 1e-12)
        dsq = small.tile([P, MT], f32, tag="dsq")
        nc.scalar.activation(dsq, d2, AF.Sqrt)
        # one Newton step: d = 0.5*(y + x/y)  (ACT sqrt table is low precision)
        rc = small.tile([P, MT], f32, tag="rc")
        nc.vector.reciprocal(rc, dsq)
        xy = small.tile([P, MT], f32, tag="xy")
        nc.vector.tensor_mul(xy, rc, d2)
        dv = small.tile([P, MT], f32, tag="dv")
        nc.vector.tensor_add(dv, dsq, xy)
        nc.vector.tensor_scalar_mul(dv, dv, 0.5)

        # ---- stats: [smax, -smin, dmax, -dmin] ----
        dneg = small.tile([P, MT], f32, tag="dneg")
        nc.vector.tensor_scalar_mul(dneg, dv, -1.0)
        nc.vector.tensor_reduce(out=STp[:, 2:3], in_=dv, axis=AX.X, op=OP.max)
        nc.vector.tensor_reduce(out=STp[:, 3:4], in_=dneg, axis=AX.X, op=OP.max)

        # partition-axis max of STp via PE transpose
        stps = psums.tile([4, P], f32, tag="ps")
        nc.tensor.transpose(stps, STp, idm32)
        STr = small.tile([4, 1], f32, tag="STr")
        nc.vector.tensor_reduce(out=STr, in_=stps, axis=AX.X, op=OP.max)

        nc.sync.dma_start(out=cc_in[:], in_=STr)
        nc.gpsimd.collective_compute(
            "AllReduce", OP.max,
            replica_groups=[list(range(CORES))],
            ins=[cc_in[:]], outs=[cc_out[:]],
        )
        G = small.tile([P, 4], f32, tag="G")
        _co = cc_out[:]
        nc.sync.dma_start(out=G, in_=bass.AP(
            tensor=_co.tensor, offset=_co.offset,
            ap=[[0, P]] + [list(d) for d in _co.ap]))

        # spans + reciprocals
        su = small.tile([P, 1], f32, tag="su")
        nc.vector.tensor_add(su, G[:, 0:1], G[:, 1:2])     # smax - smin
        nc.vector.tensor_scalar_add(su, su, EPS)
        ru = small.tile([P, 1], f32, tag="ru")
        nc.vector.reciprocal(ru, su)
        nc.vector.tensor_scalar_mul(ru, ru, -lam)          # -(lambda)/(span_u)
        sd = small.tile([P, 1], f32, tag="sd")
        nc.vector.tensor_add(sd, G[:, 2:3], G[:, 3:4])     # dmax - dmin
        nc.vector.tensor_scalar_add(sd, sd, EPS)
        rd = small.tile([P, 1], f32, tag="rd")
        nc.vector.reciprocal(rd, sd)

        # out = lam*(smax - S)/span_u + (d - dmin)/span_d
        t1 = small.tile([P, MT], f32, tag="t1")
        nc.vector.tensor_scalar(
            out=t1, in0=S, scalar1=G[:, 0:1], scalar2=ru,
            op0=OP.subtract, op1=OP.mult)
        t2 = small.tile([P, MT], f32, tag="t2")
        nc.vector.tensor_scalar(
            out=t2, in0=dv, scalar1=G[:, 3:4], scalar2=rd,
            op0=OP.add, op1=OP.mult)
        ov = small.tile([P, MT], f32, tag="ov")
        nc.vector.tensor_add(ov, t1, t2)
        nc.sync.dma_start(out=out_h[:], in_=ov)

    _dedupe_ldweights(nc)
    if legalize:
        _split_multi_waits(nc, mybir)
    return nc


def _dedupe_ldweights(nc):
    """Consecutive PE matmuls over the same stationary tile each get their
    own InstLdweights from tile_legalize; the array state is unchanged, so
    drop the repeats (moving their sync info to the next PE instruction)."""
    import concourse.mybir as mybir
    PE = mybir.EngineType.PE
    for func in nc.m.functions:
        for block in func.blocks:
            out = []
            changed = False
            last_key = None
            pending = []            # sync entries from dropped LDWs
            for inst in block.instructions:
                if inst.engine != PE:
                    out.append(inst)
                    continue
                if isinstance(inst, mybir.InstLdweights):
                    key = str(inst.ins)
                    if key == last_key:
                        si = inst.sync_info
                        if si is not None:
                            pending.extend(list(si.on_wait or []))
                            pending.extend(
                                ("upd", u) for u in (si.on_update or []))
                        changed = True
                        continue
                    last_key = key
                if pending:
                    si = inst.sync_info
                    waits = list(si.on_wait or []) if si is not None else []
                    upds = list(si.on_update or []) if si is not None else []
                    for p in pending:
                        if isinstance(p, tuple):
                            upds.append(p[1])
                        else:
                            waits.append(p)
                    inst.sync_info = mybir.SyncInfo(on_wait=waits, on_update=upds)
                    pending = []
                out.append(inst)
            if changed:
                block.instructions = out


def _split_multi_waits(nc, mybir):
    """This walrus build accepts at most ONE sync-wait command per
    instruction; Tile freely attaches several.  Hoist all but the last
    wait onto dedicated same-engine NoOps inserted just before."""
    n = 0
    for func in nc.m.functions:
        for block in func.blocks:
            out = []
            changed = False
            for inst in block.instructions:
                si = inst.sync_info
                waits = list(si.on_wait) if si is not None and si.on_wait else []
                if len(waits) > 1:
                    for w in waits[:-1]:
                        nop = mybir.InstNoOp(name=f"WSPLIT-{n}", ins=[], outs=[])
                        n += 1
                        nop.engine = inst.engine
                        nop.sync_info = mybir.SyncInfo(on_wait=[w], on_update=[])
                        out.append(nop)
                    inst.sync_info = mybir.SyncInfo(
                        on_wait=[waits[-1]],
                        on_update=list(si.on_update or []))
                    changed = True
                out.append(inst)
            if changed:
                block.instructions = out


def _prep_inputs(pred, U_z, L_z):
    f = np.float32
    h = np.float16
    pred = np.asarray(pred, dtype=f)
    U = np.asarray(U_z, dtype=f)
    L = np.asarray(L_z, dtype=f)
    lt = np.ascontiguousarray(L.T.astype(h))             # [NZ, N_L] fp16
    l2 = (L * L).sum(axis=1).astype(f)                   # [N_L]
    l2bias = np.ascontiguousarray(
        (np.float32(L2C) - l2).reshape(NCH, P).T)        # [P, NCH]
    idm = np.eye(P, dtype=h)
    in_maps = []
    for c in range(CORES):
        r = slice(c * MU, (c + 1) * MU)
        Uc = U[r]
        in_maps.append({
            "ut": np.ascontiguousarray((-2.0 * Uc).T.astype(h)),  # [NZ, MU]
            "lt": lt,
            "l2b": l2bias,
            "u2c": np.ascontiguousarray(
                ((Uc * Uc).sum(axis=1).astype(f) + np.float32(L2C))
                .reshape(MT, P).T),
            "idm": idm,
            "idm32": np.eye(P, dtype=f),
            "pred": np.ascontiguousarray(pred[r]),
        })
    return in_maps


def _run(pred, U_z, L_z, lambda_, trace=False):
    from concourse import bass_utils
    lam = float(lambda_)
    key = lam
    if key not in _CACHE:
        _CACHE[key] = _build(lam)
    nc = _CACHE[key]
    in_maps = _prep_inputs(pred, U_z, L_z)
    res = bass_utils.run_bass_kernel_spmd(
        nc, in_maps, list(range(CORES)), trace=trace)
    out = np.empty(N_U, dtype=np.float32)
    for c in range(CORES):
        ov = res.results[c]["outv"]                      # [P, MT]
        out[c * MU:(c + 1) * MU] = ov.T.reshape(MU)
    return out, res


def kernel(pred, U_z, L_z, lambda_):
    out, _ = _run(pred, U_z, L_z, lambda_)
    return out
